# revision 12
# baseline (speedup 1.0000x reference)
"""Causal self-attention (B=2, T=4096, C=768, H=12, D=64) on 8 trn2 cores.

Sharding: batch*heads across cores. Core c handles batch c//4 and heads
3*(c%4) .. 3*(c%4)+2. Each core computes the QKV projection for its head
slice, full causal attention for those heads, and a partial output
projection (its heads' rows of w_out). The host sums the 4 partials per
batch and adds b_out.

On-core layouts (matmul operands float32r - fp32 data consumed at full
PE rate with ~1e-4 rounding; the PE rounds on read, so fp32 bits are
DMA'd straight into f32r tiles):
  xT      [C, T]   input, pre-transposed on host
  qT/kT   [64, T]  packed so q_h and k_h share a partition base
                   (matmul requires lhsT/rhs base alignment)
  v_aug   [T, 256] natural-layout v with a ones column per head at
                   col h*65+64 (so P@V also yields softmax denominators)
  scoresT [k, q]   psum; exp on ACT; causal mask via gpsimd affine_select
  outT    [65, q]  psum accumulation over k tiles; row 64 = sum(exp)

Packed [128, T] sbuf tiles (rows 0:64 | 64:128):
  tA = [qT_h0 | qT_h1]   tB = [kT_h0 | kT_h1]
  tC = [outT_h0 | qT_h2] tD = [outT_h1 | kT_h2] tE = [outT_h2 | -]
"""

import numpy as np

import concourse.bass as bass
import concourse.mybir as mybir
import concourse.tile as tile
from concourse import bacc
from concourse.bass_utils import run_bass_kernel_spmd

B, T, C = 2, 4096, 768
NH, D = 12, 64
HPC = 3  # heads per core
NCORES = 8
P = 128
TCH = 256          # phase-1 T chunk
NT1 = T // TCH     # 16
NKT = T // P       # 32 k tiles
QB = 512           # q block
NQB = T // QB      # 8
F32 = mybir.dt.float32
F32R = mybir.dt.float32r

_CACHE = {}


def _build_nc():
    nc = bacc.Bacc(
        "TRN2",
        target_bir_lowering=False,
        debug=False,
        enable_asserts=False,
        num_devices=NCORES,
    )
    # wqk columns: [q_h0 q_h1 | k_h0 k_h1 | q_h2 | k_h2]
    xT = nc.dram_tensor("xT", [C, T], F32R, kind="ExternalInput")
    wqk = nc.dram_tensor("wqk", [C, 2 * HPC * D], F32R, kind="ExternalInput")
    wv = nc.dram_tensor("wv", [C, 256], F32R, kind="ExternalInput")
    wo = nc.dram_tensor("wo", [HPC * D, C], F32R, kind="ExternalInput")
    out = nc.dram_tensor("out", [T, C], F32, kind="ExternalOutput")

    with tile.TileContext(nc) as tc:
        _emit(tc, nc, xT.ap(), wqk.ap(), wv.ap(), wo.ap(), out.ap())
    nc.compile()
    return nc


def _emit(tc, nc, xT, wqk, wv, wo, out):
    import contextlib

    ctx = contextlib.ExitStack()
    with ctx:
        # ---- persistent sbuf ----
        persist = ctx.enter_context(tc.tile_pool(name="persist", bufs=1))
        packs = [
            persist.tile([P, T], F32R, tag=f"pk{m}", name=f"pk{m}") for m in range(5)
        ]
        tA, tB, tC, tD, tE = packs
        vaug = persist.tile([P, NKT, 256], F32R, tag="vaug")
        wqk_sb = persist.tile([P, 6, 2 * HPC * D], F32R, tag="wqk")
        wv_sb = persist.tile([P, 6, 256], F32R, tag="wv")
        wo_sb = [
            persist.tile([D, C], F32R, tag=f"wo{h}", name=f"wo{h}")
            for h in range(HPC)
        ]
        ones_r = persist.tile([P, D], F32R, tag="ones_r")
        ones_f32 = persist.tile([P, D], F32, tag="onesf32")

        nc.sync.dma_start(out=wqk_sb[:], in_=wqk.rearrange("(co p) n -> p co n", p=P))
        nc.sync.dma_start(out=wv_sb[:], in_=wv.rearrange("(co p) n -> p co n", p=P))
        for h in range(HPC):
            nc.sync.dma_start(out=wo_sb[h][:], in_=wo[h * D : (h + 1) * D, :])
        nc.gpsimd.memset(ones_f32[:], 1.0)
        nc.vector.tensor_copy(out=ones_r[:], in_=ones_f32[:])

        def qT(h):
            return (tA[0:D], tA[D:P], tC[D:P])[h]

        def kT(h):
            return (tB[0:D], tB[D:P], tD[D:P])[h]

        def outT(h):
            return (tC[0:D], tD[0:D], tE[0:D])[h]

        # ---- phase 1: QKV projection ----
        # proj chains: m0 -> tA (full), m1 -> tB (full),
        # m2 = q_h2 (M=64, psum base 64) -> tC[64:128],
        # m3 = k_h2 -> tD[64:128]
        with (
            tc.tile_pool(name="xchunks", bufs=2) as xpool,
            tc.tile_pool(name="p1psum", bufs=4, space="PSUM") as p1psum,
            tc.tile_pool(name="p1vpsum", bufs=2, space="PSUM") as p1vpsum,
        ):
            for tch in range(NT1):
                sl = slice(tch * TCH, (tch + 1) * TCH)
                xt = xpool.tile([P, 6, TCH], F32R, tag="xt")
                nc.sync.dma_start(
                    out=xt[:], in_=xT[:, sl].rearrange("(co p) t -> p co t", p=P)
                )
                for ci in range(3):
                    ps = p1psum.tile([P, TCH], F32, tag="p1", name=f"p1_{tch}_{ci}")
                    for c6 in range(6):
                        nc.tensor.matmul(
                            ps[:],
                            wqk_sb[:, c6, ci * P : (ci + 1) * P],
                            xt[:, c6, :],
                            start=(c6 == 0),
                            stop=(c6 == 5),
                        )
                    if ci < 2:
                        dst = (tA, tB)[ci]
                        nc.vector.tensor_copy(out=dst[:, sl], in_=ps[:])
                    else:
                        # chain 2 = [qT_h2 | kT_h2] at psum base 0; the packed
                        # destinations live at partition base 64, which only a
                        # DMA can reach (engines cannot cross partitions)
                        stg = xpool.tile([P, TCH], F32R, tag="stg")
                        nc.vector.tensor_copy(out=stg[:], in_=ps[:])
                        nc.sync.dma_start(out=tC[D:P, sl], in_=stg[0:D, :])
                        nc.sync.dma_start(out=tD[D:P, sl], in_=stg[D:P, :])
                for half in range(2):
                    ktv = tch * 2 + half
                    ps2 = p1vpsum.tile([P, 256], F32, tag="p1v")
                    for c6 in range(6):
                        nc.tensor.matmul(
                            ps2[:],
                            xt[:, c6, half * P : (half + 1) * P],
                            wv_sb[:, c6, :],
                            start=(c6 == 0),
                            stop=(c6 == 5),
                        )
                    nc.vector.tensor_copy(out=vaug[:, ktv, :], in_=ps2[:])
            # ones columns for the softmax-denominator trick
            for h in range(HPC):
                nc.vector.tensor_copy(
                    out=vaug[:, :, h * (D + 1) + D], in_=ones_f32[:, 0:NKT]
                )

        # ---- phase 2: attention ----
        with (
            tc.tile_pool(name="spsum", bufs=2, space="PSUM") as spool,
            tc.tile_pool(name="opsum", bufs=2, space="PSUM") as opool,
            tc.tile_pool(name="bpsum", bufs=1, space="PSUM") as bpool,
            tc.tile_pool(name="exps", bufs=2) as epool,
            tc.tile_pool(name="smalls", bufs=4) as rpool,
        ):
            for qb in range(NQB):
                qsl = slice(qb * QB, (qb + 1) * QB)
                for h in range(HPC):
                    nkt = 4 * qb + 4
                    outp = opool.tile([D + 1, QB], F32, tag="outT")
                    for g in range((nkt + 1) // 2):
                        kts = [kt for kt in (2 * g, 2 * g + 1) if kt < nkt]
                        sp = spool.tile([P, 2, QB], F32, tag="scores")
                        for j, kt in enumerate(kts):
                            nc.tensor.matmul(
                                sp[:, j, :],
                                kT(h)[:, kt * P : (kt + 1) * P],
                                qT(h)[:, qsl],
                                start=True,
                                stop=True,
                            )
                        ex = epool.tile([P, 2, QB], F32R, tag="ex")
                        nc.scalar.activation(
                            out=ex[:, : len(kts), :],
                            in_=sp[:, : len(kts), :],
                            func=mybir.ActivationFunctionType.Exp,
                            scale=float(D) ** -0.5,
                        )
                        for j, kt in enumerate(kts):
                            if kt >= 4 * qb:  # diagonal band: causal mask
                                nc.gpsimd.affine_select(
                                    out=ex[:, j, :],
                                    in_=ex[:, j, :],
                                    compare_op=mybir.AluOpType.is_ge,
                                    fill=0.0,
                                    base=-P * (kt - 4 * qb),
                                    pattern=[[1, QB]],
                                    channel_multiplier=-1,
                                )
                            nc.tensor.matmul(
                                outp[:],
                                vaug[:, kt, h * (D + 1) : (h + 1) * (D + 1)],
                                ex[:, j, :],
                                start=(kt == 0),
                                stop=(kt == nkt - 1),
                            )
                    # softmax denominators: reciprocal of outp row 64 stays at
                    # partition base 64 (engines cannot cross partitions), then
                    # a ones-matmul broadcasts it across partitions 0:64
                    recip = rpool.tile([D + 1, QB], F32R, tag="recip")
                    with nc.allow_low_precision(reason="f32r recip, ~1e-4"):
                        nc.vector.reciprocal(
                            out=recip[D : D + 1, :], in_=outp[D : D + 1, :]
                        )
                    bc = bpool.tile([D, QB], F32, tag="bcast")
                    nc.tensor.matmul(
                        bc[:],
                        ones_r[D : D + 1, :],
                        recip[D : D + 1, :],
                        start=True,
                        stop=True,
                    )
                    bcs = rpool.tile([D, QB], F32, tag="bcs")
                    nc.vector.tensor_copy(out=bcs[:], in_=bc[:])
                    nc.vector.tensor_mul(
                        out=outT(h)[:, qsl], in0=outp[0:D, :], in1=bcs[:]
                    )

        # ---- phase 3: output projection (partial) ----
        with (
            tc.tile_pool(name="p3psum", bufs=4, space="PSUM") as p3psum,
            tc.tile_pool(name="p3sb", bufs=3) as p3sb,
        ):
            for tt in range(T // P):
                tsl = slice(tt * P, (tt + 1) * P)
                so = p3sb.tile([P, C], F32, tag="p3out")
                for noff, nsz in ((0, 512), (512, 256)):
                    po = p3psum.tile([P, 512], F32, tag="p3")
                    for h in range(HPC):
                        nc.tensor.matmul(
                            po[:, :nsz],
                            outT(h)[:, tsl],
                            wo_sb[h][:, noff : noff + nsz],
                            start=(h == 0),
                            stop=(h == HPC - 1),
                        )
                    nc.vector.tensor_copy(
                        out=so[:, noff : noff + nsz], in_=po[:, :nsz]
                    )
                nc.sync.dma_start(out=out[tsl, :], in_=so[:])


def _get_nc():
    if "nc" not in _CACHE:
        _CACHE["nc"] = _build_nc()
    return _CACHE["nc"]


def _shard_inputs(x, w_qkv, w_out):
    """Build per-core input maps."""
    x = np.asarray(x, dtype=np.float32)
    w_qkv = np.asarray(w_qkv, dtype=np.float32)
    w_out = np.asarray(w_out, dtype=np.float32)
    xTs = [np.ascontiguousarray(x[b].T) for b in range(B)]
    in_maps = []
    for c in range(NCORES):
        b = c // 4
        heads = [HPC * (c % 4) + i for i in range(HPC)]
        q = [w_qkv[:, h * D : (h + 1) * D] for h in heads]
        k = [w_qkv[:, C + h * D : C + (h + 1) * D] for h in heads]
        wqk = np.concatenate([q[0], q[1], k[0], k[1], q[2], k[2]], axis=1)
        wv = np.zeros((C, 256), dtype=np.float32)
        for i, h in enumerate(heads):
            wv[:, i * (D + 1) : i * (D + 1) + D] = w_qkv[
                :, 2 * C + h * D : 2 * C + (h + 1) * D
            ]
        wo = np.concatenate(
            [w_out[h * D : (h + 1) * D, :] for h in heads], axis=0
        )
        in_maps.append(
            {
                "xT": xTs[b],
                "wqk": np.ascontiguousarray(wqk),
                "wv": wv,
                "wo": np.ascontiguousarray(wo),
            }
        )
    return in_maps


def kernel(x, w_qkv, w_out, b_out):
    nc = _get_nc()
    in_maps = _shard_inputs(x, w_qkv, w_out)
    res = run_bass_kernel_spmd(nc, in_maps, core_ids=list(range(NCORES)))
    b_out = np.asarray(b_out, dtype=np.float32)
    outs = []
    for b in range(B):
        acc = res.results[4 * b]["out"].astype(np.float32).copy()
        for c in range(4 * b + 1, 4 * b + 4):
            acc += res.results[c]["out"]
        outs.append(acc + b_out[None, :])
    return np.stack(outs, axis=0)
